# revision 57
# baseline (speedup 1.0000x reference)
"""Causal self-attention (B=8, T=1024, C=768, H=12) on 8 TRN2 NeuronCores.

Data-parallel over batch: each core computes one batch element end-to-end.
No collectives.

Per-core kernel design (cost-model driven):
  - QKV and output projections run as 3-stream hi/lo fp8-e4m3 DoubleRow
    matmuls (xh@Wh + xh@Wl + xl@Wh): K=256 per instruction at 0.5
    cycles/row -> ~4x the fp32r/bf16 GEMM rate. Weights are scaled by 32
    so lo-plane residuals clear the e4m3 subnormal floor; the 1/1024
    compensation folds into the exp scale and the final output copy.
  - S = K^T Q stays bf16 [keys, queries] with exact causal column counts.
  - PV is flipped to [queries, dk] (M fully packed) in bf16 with a ones
    column appended to V for the softmax denominators; normalization is a
    per-partition divide, then a PE transpose back to feature-major for
    the output projection.
  - Weights are pre-split/laid out on the host; engine work is balanced
    across Act (exp), DVE (copies/masks) and Pool (converts/normalize).

Self-contained: builds and compiles the Bass program on first call and
caches it for subsequent calls.
"""

import numpy as np
import ml_dtypes

import concourse.bass as bass
import concourse.mybir as mybir
from concourse import bacc
from concourse.tile import TileContext
from concourse.bass_utils import run_bass_kernel_spmd
from concourse.masks import make_identity, make_upper_triangular

f32 = mybir.dt.float32
f32r = mybir.dt.float32r
bf16 = mybir.dt.bfloat16
e4 = mybir.dt.float8e4
u8 = mybir.dt.uint8
EXP = mybir.ActivationFunctionType.Exp
COPY = mybir.ActivationFunctionType.Copy
DIV = mybir.AluOpType.divide
DR = mybir.MatmulPerfMode.DoubleRow
E4NP = ml_dtypes.float8_e4m3

N_CORES = 8
T = 1024
C = 768
H = 12
DK = 64
NTT = 8            # token tiles of 128
NC2 = 3            # K-chunk pairs of 256
SW = 32.0          # weight scale (power of 2)
SCALE_EXP = (DK ** -0.5) / (SW * SW)
SCALE_OUT = 1.0 / (SW * SW)
STREAMS = ((0, 0), (0, 1), (1, 0))   # (x-plane, w-plane) hi/lo cross terms

# per-head pb column offsets: block ki covers queries [128*ki, 1024)
PB_OFF = [0]
for _ki in range(NTT):
    PB_OFF.append(PB_OFF[-1] + (T - 128 * _ki))
PB_W = PB_OFF[-1]  # 4608


def build_program(qkv_bias: bool, out_bias: bool):
    nc = bacc.Bacc("TRN2", num_devices=N_CORES, debug=False)

    x = nc.dram_tensor("x", [T, C], f32, kind="ExternalInput").ap()
    wq_d = [[nc.dram_tensor(f"wq_{s}{c2}", [128, 2 * 3 * C], u8,
                            kind="ExternalInput").ap()
             for c2 in range(NC2)] for s in range(2)]
    wo_d = [[nc.dram_tensor(f"wo_{s}{c2}", [128, 2 * C], u8,
                            kind="ExternalInput").ap()
             for c2 in range(NC2)] for s in range(2)]
    on_d = nc.dram_tensor("ones12", [128, H], u8 if False else bf16,
                          kind="ExternalInput").ap()
    bq_d = nc.dram_tensor("b_qkv", [3 * C], f32, kind="ExternalInput").ap()
    bo_d = nc.dram_tensor("b_out", [C], f32, kind="ExternalInput").ap()
    y = nc.dram_tensor("y", [T, C], f32, kind="ExternalOutput").ap()

    with TileContext(nc) as tc:
        with tc.tile_pool(name="const", bufs=1) as cpool, \
             tc.tile_pool(name="psn", bufs=1, space="PSUM") as psn:

            ident = cpool.tile([128, 128], bf16, tag="ident")
            nc.gpsimd.memset(ident.bitcast(f32), 0.0)
            make_identity(nc, ident, nomemset=True)
            identf = cpool.tile([128, 128], f32r, tag="identf")
            nc.gpsimd.memset(identf.bitcast(f32), 0.0)
            make_identity(nc, identf, nomemset=True)
            tri = cpool.tile([128, 128], bf16, tag="tri")
            nc.gpsimd.memset(tri.bitcast(f32), 0.0)
            nc.gpsimd.affine_select(
                out=tri, in_=tri, compare_op=mybir.AluOpType.is_gt,
                fill=1.0, base=0, pattern=[[-1, 128]], channel_multiplier=1)

            # ---- persistent SBUF tiles
            wq8 = [[cpool.tile([128, 2 * 3 * C], u8, tag=f"wq{s}{c2}",
                               name=f"wq{s}{c2}")
                    for c2 in range(NC2)] for s in range(2)]
            wo8 = [[cpool.tile([128, 2 * C], u8, tag=f"wo{s}{c2}",
                               name=f"wo{s}{c2}")
                    for c2 in range(NC2)] for s in range(2)]
            xT8 = [cpool.tile([128, 6, T], e4, tag=f"xT8{s}", name=f"xT8{s}")
                   for s in range(2)]
            qkT = [cpool.tile([128, T], bf16, tag=f"qkT{m}", name=f"qkT{m}")
                   for m in range(12)]
            vp = [cpool.tile([128, H, 65], bf16, tag=f"vp{t}", name=f"vp{t}")
                  for t in range(NTT)]
            at8 = [[cpool.tile([128, 2 * T], e4, tag=f"at8{s}{c2}",
                               name=f"at8{s}{c2}")
                    for c2 in range(NC2)] for s in range(2)]

            # named PSUM tiles (manually rotated sub-regions)
            pspv = psn.tile([128, 260], f32, tag="pspv")   # 2 x [*,130] qi slots
            pst = psn.tile([128, 1024], bf16, tag="pst")   # 2 x [*,512] qj slots

            def wqv(s, c2):
                return wq8[s][c2].bitcast(e4).rearrange("p (i m) -> p i m", i=2)

            def wov(s, c2):
                return wo8[s][c2].bitcast(e4).rearrange("p (i m) -> p i m", i=2)

            def atv(s, c2):
                return at8[s][c2].rearrange("p (i m) -> p i m", i=2)

            def x8(s, c2, a, b):
                return xT8[s][:, 2 * c2:2 * c2 + 2, a:b]

            if qkv_bias or out_bias:
                ones_bf = cpool.tile([1, 512], bf16, tag="ones_bf")
                nc.gpsimd.memset(ones_bf, 1.0)
            if qkv_bias:
                bqf = cpool.tile([1, 3 * C], f32, tag="bqf")
                nc.sync.dma_start(out=bqf, in_=bq_d[None, :])
                bqb = cpool.tile([1, 3 * C], bf16, tag="bqb")
                nc.vector.tensor_scalar_mul(bqb, bqf, SW)
            if out_bias:
                bof = cpool.tile([1, C], f32, tag="bof")
                nc.sync.dma_start(out=bof, in_=bo_d[None, :])
                bob = cpool.tile([1, C], bf16, tag="bob")
                nc.vector.tensor_scalar_mul(bob, bof, SW * SW)

            # ---- x load -> PE transpose (f32r) -> fp8 hi/lo split
            # Wv slices ([2C:3C] of each plane) DMA'd on the Act queue,
            # interleaved with x-odd loads; Wqk+Wo on SP after x-evens.
            with tc.tile_pool(name="xst", bufs=3) as xst, \
                 tc.tile_pool(name="xtp", bufs=2, space="PSUM") as xtpp, \
                 tc.tile_pool(name="psv", bufs=2, space="PSUM") as psv:
                for t in range(NTT):
                    ts0, ts1 = t * 128, (t + 1) * 128
                    xs = xst.tile([128, C], f32r, tag="xs")
                    # half-column loads across both DMA queues; Wv plane
                    # DMAs right after the first tile on the Act queue
                    if t == 0:
                        nc.sync.dma_start(out=xs[:, 0:128],
                                          in_=x[ts0:ts1, 0:128].bitcast(f32r))
                        nc.sync.dma_start(out=xs[:, 128:384],
                                          in_=x[ts0:ts1, 128:384].bitcast(f32r))
                    else:
                        nc.sync.dma_start(out=xs[:, 0:384],
                                          in_=x[ts0:ts1, 0:384].bitcast(f32r))
                    xbq = nc.gpsimd if t == 0 else nc.scalar
                    xbq.dma_start(out=xs[:, 384:768],
                                  in_=x[ts0:ts1, 384:768].bitcast(f32r))
                    if t == 0:
                        engs = [nc.sync, nc.scalar, nc.gpsimd]
                        for c2 in range(NC2):
                            for s in range(2):
                                wv3 = wq_d[s][c2].rearrange(
                                    "p (i m) -> p i m", i=2)
                                dst = wq8[s][c2].rearrange(
                                    "p (i m) -> p i m", i=2)[:, :, 2 * C:3 * C]
                                engs[(2 * c2 + s) % 3].dma_start(
                                    out=dst, in_=wv3[:, :, 2 * C:3 * C])
                    for hf in range(2):
                        xtp = xtpp.tile([128, 384], f32r, tag="xtp")
                        for j in range(3):
                            c = 3 * hf + j
                            nc.tensor.transpose(
                                xtp[:, j * 128:(j + 1) * 128],
                                xs[:, c * 128:(c + 1) * 128], identf)
                        src = xtp.bitcast(f32).rearrange(
                            "p (c w) -> p c w", w=128)
                        hi = xT8[0][:, 3 * hf:3 * hf + 3, ts0:ts1]
                        nc.scalar.activation(hi, src, COPY)
                        nc.vector.tensor_sub(
                            xT8[1][:, 3 * hf:3 * hf + 3, ts0:ts1], src, hi)

                    # V'(t) = x(t) @ Wv (3-stream fp8 DR), token-major
                    pv = psv.tile([128, C], f32, tag="pv")
                    for n0, nw in ((0, 512), (512, 256)):
                        first = True
                        if qkv_bias:
                            nc.tensor.matmul(
                                pv[:, n0:n0 + nw], ones_bf[0:1, 0:128],
                                bqb[0:1, 2 * C + n0:2 * C + n0 + nw],
                                start=True, stop=False)
                            first = False
                        for c2 in range(NC2):
                            for sa, sb in STREAMS:
                                last = c2 == NC2 - 1 and (sa, sb) == (1, 0)
                                nc.tensor.matmul(
                                    pv[:, n0:n0 + nw],
                                    x8(sa, c2, ts0, ts1),
                                    wqv(sb, c2)[:, :, 2 * C + n0:
                                                2 * C + n0 + nw],
                                    start=first, stop=last, perf_mode=DR)
                                first = False
                    pvv = pv.rearrange("p (h e) -> p h e", e=64)
                    nc.vector.tensor_copy(vp[t][:, 0:6, 0:64], pvv[:, 0:6, :])
                    nc.scalar.activation(vp[t][:, 6:12, 0:64], pvv[:, 6:12, :],
                                         COPY)

                # Wqk on the Pool (swdge) queue to keep SP/Act free for x
                for c2 in range(NC2):
                    for s in range(2):
                        wv3 = wq_d[s][c2].rearrange("p (i m) -> p i m", i=2)
                        dst = wq8[s][c2].rearrange(
                            "p (i m) -> p i m", i=2)[:, :, 0:2 * C]
                        nc.gpsimd.dma_start(out=dst, in_=wv3[:, :, 0:2 * C])
                # ones columns of V' (softmax denominators); DMA'd because
                # hardware memset cannot write 2-byte value types
                for t in range(NTT):
                    nc.gpsimd.dma_start(out=vp[t][:, :, 64:65],
                                        in_=on_d.rearrange("p (h o) -> p h o",
                                                           o=1))

            # ---- Q,K + attention, interleaved per head pair
            with tc.tile_pool(name="psqk", bufs=2, space="PSUM") as psqkp, \
                 tc.tile_pool(name="pss", bufs=2, space="PSUM") as pss, \
                 tc.tile_pool(name="pb", bufs=2) as pbp, \
                 tc.tile_pool(name="pvs", bufs=6) as pvsp, \
                 tc.tile_pool(name="rc", bufs=6) as rcp, \
                 tc.tile_pool(name="at", bufs=6) as atp:
                carried = []
                for m in range(6):
                    # q features (m), k features (6+m)
                    def emit_qk(mm, nj):
                        pq = psqkp.tile([128, 512], f32, tag="pq", name="pq")
                        first = True
                        if qkv_bias:
                            nc.tensor.matmul(
                                pq, bqb[0:1, mm * 128:(mm + 1) * 128],
                                ones_bf, start=True, stop=False)
                            first = False
                        for c2 in range(NC2):
                            for sa, sb in STREAMS:
                                last = (c2 == NC2 - 1
                                        and (sa, sb) == (1, 0))
                                nc.tensor.matmul(
                                    pq,
                                    wqv(sb, c2)[:, :, mm * 128:(mm + 1) * 128],
                                    x8(sa, c2, nj * 512, (nj + 1) * 512),
                                    start=first, stop=last, perf_mode=DR)
                                first = False
                        qeng = nc.vector if nj == 0 else nc.scalar
                        if qeng is nc.vector:
                            qeng.tensor_copy(
                                qkT[mm][:, nj * 512:(nj + 1) * 512], pq)
                        else:
                            nc.scalar.activation(
                                qkT[mm][:, nj * 512:(nj + 1) * 512], pq, COPY)

                    if m == 0:      # pair 0: nj=0 q/k before attention
                        emit_qk(0, 0)
                        emit_qk(6, 0)
                    if m == 1:      # Wout loads queue behind Wqk on Pool
                        for c2o in range(NC2):
                            for so in range(2):
                                nc.gpsimd.dma_start(out=wo8[so][c2o],
                                                    in_=wo_d[so][c2o])
                    # qj=0 carries this pair's nj=1 q/k groups (needed only
                    # by qj=1); qj=1 carries the next pair's nj=0 groups.
                    # Both act as PE filler while Act runs exp.
                    nxt0 = [(m, 1), (6 + m, 1)]
                    nxt1 = [(m + 1, 0), (7 + m, 0)] if m < 5 else []

                    # attention for head pair (2m, 2m+1)
                    pbt = pbp.tile([128, 2 * PB_W], bf16, tag="pbt",
                                   name=f"pb{m}")
                    qT, kT = qkT[m], qkT[6 + m]
                    for qj in range(2):
                        nxt = nxt0 if qj == 0 else nxt1
                        for ki in range(4 * qj + 4):
                            if ki % 2 == 1 and nxt:
                                emit_qk(*nxt.pop(0))
                            if m == 5 and qj == 1 and ki in (1, 3):
                                # last pair has no next-pair q/k filler:
                                # start out-proj t=0 groups over c2 0..1
                                n0o = 0 if ki == 1 else 512
                                og = psqkp.tile([128, 512], f32, tag="pq",
                                                name=f"og{ki}")
                                ofirst = True
                                if out_bias:
                                    nc.tensor.matmul(
                                        og[:, 0:512 if n0o == 0 else 256],
                                        ones_bf[0:1, 0:128],
                                        bob[0:1, n0o:n0o + (512 if n0o == 0
                                                            else 256)],
                                        start=True, stop=False)
                                    ofirst = False
                                nwo = 512 if n0o == 0 else 256
                                for c2o in range(2):
                                    for sa, sb in STREAMS:
                                        nc.tensor.matmul(
                                            og[:, 0:nwo],
                                            atv(sa, c2o)[:, :, 0:128],
                                            wov(sb, c2o)[:, :, n0o:n0o + nwo],
                                            start=ofirst, stop=False,
                                            perf_mode=DR)
                                        ofirst = False
                                carried.append((og, n0o, nwo))
                            o = max(0, 128 * ki - 512 * qj)
                            ps = pss.tile([128, 1024], f32, tag="s",
                                          name=f"s{m}_{qj}_{ki}")
                            for e in range(2):
                                nc.tensor.matmul(
                                    ps[:, e * 512 + o:(e + 1) * 512],
                                    kT[64 * e:64 * e + 64,
                                       ki * 128:(ki + 1) * 128],
                                    qT[64 * e:64 * e + 64,
                                       qj * 512 + o:(qj + 1) * 512],
                                    start=True, stop=True)
                            # exp both heads in one strided op
                            wv = 512 - o
                            rel = PB_OFF[ki] + 512 * qj + o - 128 * ki
                            src = ps.rearrange(
                                "p (e w) -> p e w", w=512)[:, :, o:512]
                            dst = pbt.rearrange(
                                "p (e w) -> p e w", w=PB_W)[:, :, rel:rel + wv]
                            nc.scalar.activation(dst, src, EXP,
                                                 scale=float(SCALE_EXP))
                            if ki >= 4 * qj:
                                # diagonal block: causal mask
                                for e in range(2):
                                    dg = pbt[:, e * PB_W + PB_OFF[ki]:
                                             e * PB_W + PB_OFF[ki] + 128]
                                    eng = nc.gpsimd if e == 0 else nc.vector
                                    eng.tensor_mul(dg, dg, tri)
                                # PV(qi=ki) is unblocked as soon as this
                                # block's exp+mask lands: emit it here so
                                # the PE interleaves PV with the next S
                                qi = ki
                                base = (qi % 2) * 130
                                half = ((2 * m + qj) % 2) * 512
                                for e in range(2):
                                    for kj in range(qi + 1):
                                        lo = PB_OFF[kj] + 128 * (qi - kj)
                                        nc.tensor.matmul(
                                            pspv[:, base + e * 65:
                                                 base + (e + 1) * 65],
                                            pbt[:, e * PB_W + lo:
                                                e * PB_W + lo + 128],
                                            vp[kj][:, 2 * m + e:2 * m + e + 1, :],
                                            start=(kj == 0), stop=(kj == qi))
                                if qi % 2 == 1:
                                    # both slots full: one copy + recip for
                                    # the qi pair, then normalize+transpose
                                    pvc = pvsp.tile([128, 260], f32,
                                                    tag="pvc")
                                    nc.vector.tensor_copy(pvc, pspv)
                                    rc = rcp.tile([128, 4], f32, tag="rc")
                                    nc.vector.reciprocal(
                                        rc.rearrange("p (e w) -> p e w", w=1),
                                        pvc.rearrange("p (e w) -> p e w",
                                                      w=65)[:, :, 64:65])
                                    for qq in (qi - 1, qi):
                                        b2 = (qq % 2) * 130
                                        at = atp.tile([128, 128], bf16,
                                                      tag="at")
                                        for e in range(2):
                                            nc.gpsimd.tensor_scalar_mul(
                                                at[:, e * 64:(e + 1) * 64],
                                                pvc[:, b2 + e * 65:
                                                    b2 + e * 65 + 64],
                                                rc[:, (qq % 2) * 2 + e:
                                                   (qq % 2) * 2 + e + 1])
                                        col = half + (qq - 4 * qj) * 128
                                        nc.tensor.transpose(
                                            pst[:, col:col + 128], at, ident)
                        half = ((2 * m + qj) % 2) * 512
                        while nxt:          # flush remaining q/k groups
                            emit_qk(*nxt.pop(0))
                        # feature-major fp8 hi/lo of this qj's attn
                        c2, i = m // 2, m % 2
                        hi = at8[0][c2][:, i * T + qj * 512:i * T + (qj + 1) * 512]
                        nc.vector.tensor_copy(hi, pst[:, half:half + 512])
                        nc.vector.tensor_sub(
                            at8[1][c2][:, i * T + qj * 512:i * T + (qj + 1) * 512],
                            pst[:, half:half + 512], hi)

                for og, n0, nw in carried:      # finish pair-5-carried t=0
                    for sa, sb in STREAMS:
                        nc.tensor.matmul(
                            og[:, 0:nw], atv(sa, 2)[:, :, 0:128],
                            wov(sb, 2)[:, :, n0:n0 + nw],
                            start=False, stop=(sa, sb) == (1, 0),
                            perf_mode=DR)
                    ys = atp.tile([128, 512], f32, tag="ysc", name="ysc")
                    nc.vector.tensor_scalar_mul(ys[:, 0:nw], og[:, 0:nw],
                                                float(SCALE_OUT))
                    deng = nc.sync if n0 == 0 else nc.scalar
                    deng.dma_start(out=y[0:128, n0:n0 + nw], in_=ys[:, 0:nw])

            # ---- output projection: y = attn' @ Wout' / 1024
            with tc.tile_pool(name="pso", bufs=3, space="PSUM") as pso, \
                 tc.tile_pool(name="yst", bufs=4) as yst:
                for t in range(NTT):
                    for n0, nw in ((0, 512), (512, 256)):
                        if t == 0 and carried:
                            continue
                        pot = pso.tile([128, 512], f32, tag="po", name="po")
                        po = pot[:, 0:nw]
                        first = True
                        if out_bias:
                            nc.tensor.matmul(
                                po, ones_bf[0:1, 0:128],
                                bob[0:1, n0:n0 + nw], start=True, stop=False)
                            first = False
                        for c2 in range(NC2):
                            for sa, sb in STREAMS:
                                last = c2 == NC2 - 1 and (sa, sb) == (1, 0)
                                nc.tensor.matmul(
                                    po, atv(sa, c2)[:, :, t * 128:(t + 1) * 128],
                                    wov(sb, c2)[:, :, n0:n0 + nw],
                                    start=first, stop=last, perf_mode=DR)
                                first = False
                        ys = yst.tile([128, 512], f32, tag="ys")
                        if t % 2 == 0:
                            nc.vector.tensor_scalar_mul(ys[:, 0:nw], po,
                                                        float(SCALE_OUT))
                        else:
                            nc.scalar.activation(ys[:, 0:nw], po, COPY,
                                                 scale=float(SCALE_OUT))
                        deng = nc.sync if (2 * t + n0 // 512) % 2 == 0 \
                            else nc.scalar
                        deng.dma_start(
                            out=y[t * 128:(t + 1) * 128, n0:n0 + nw],
                            in_=ys[:, 0:nw])

    nc.compile()
    return nc


_CACHE = {}


def _get_program(qkv_bias: bool, out_bias: bool):
    key = (qkv_bias, out_bias)
    if key not in _CACHE:
        _CACHE[key] = build_program(qkv_bias, out_bias)
    return _CACHE[key]


def prep_weights(W_qkv, W_out):
    """Host-side: scale by 32, split into e4m3 hi/lo, DoubleRow layout
    [128, 2, M] per 256-wide K chunk, as raw bytes."""
    out = {}
    for name, W, M in (("wq", np.asarray(W_qkv, np.float32), 3 * C),
                       ("wo", np.asarray(W_out, np.float32), C)):
        Ws = W * SW
        hi = Ws.astype(E4NP)
        lo = (Ws - hi.astype(np.float32)).astype(E4NP)
        for s, plane in enumerate((hi, lo)):
            a = plane.reshape(NC2, 2, 128, M)          # [c2, i, p, m]
            for c2 in range(NC2):
                lay = np.ascontiguousarray(
                    a[c2].transpose(1, 0, 2))          # [p, i, m]
                out[f"{name}_{s}{c2}"] = lay.reshape(128, 2 * M).view(np.uint8)
    return out


def _make_in_maps(x, W_qkv, b_qkv, W_out, b_out):
    x = np.ascontiguousarray(np.asarray(x, dtype=np.float32))
    b_qkv = np.ascontiguousarray(np.asarray(b_qkv, dtype=np.float32))
    b_out = np.ascontiguousarray(np.asarray(b_out, dtype=np.float32))
    w = prep_weights(W_qkv, W_out)
    w["ones12"] = np.ones((128, H), dtype=ml_dtypes.bfloat16)
    return [
        {"x": x[i], "b_qkv": b_qkv, "b_out": b_out, **w}
        for i in range(N_CORES)
    ]


def kernel(x, W_qkv, b_qkv, W_out, b_out):
    qkv_bias = bool(np.any(np.asarray(b_qkv)))
    out_bias = bool(np.any(np.asarray(b_out)))
    nc = _get_program(qkv_bias, out_bias)
    in_maps = _make_in_maps(x, W_qkv, b_qkv, W_out, b_out)
    res = run_bass_kernel_spmd(nc, in_maps, core_ids=list(range(N_CORES)))
    return np.stack([res.results[i]["y"] for i in range(N_CORES)], axis=0)


def bench(x, W_qkv, b_qkv, W_out, b_out, trace=True):
    """Run with NTFF tracing; returns (output, BassKernelResults)."""
    qkv_bias = bool(np.any(np.asarray(b_qkv)))
    out_bias = bool(np.any(np.asarray(b_out)))
    nc = _get_program(qkv_bias, out_bias)
    in_maps = _make_in_maps(x, W_qkv, b_qkv, W_out, b_out)
    res = run_bass_kernel_spmd(nc, in_maps, core_ids=list(range(N_CORES)),
                               trace=trace)
    out = np.stack([res.results[i]["y"] for i in range(N_CORES)], axis=0)
    return out, res


# revision 58
# speedup vs baseline: 1.0423x; 1.0423x over previous
"""Causal self-attention (B=8, T=1024, C=768, H=12) on 8 TRN2 NeuronCores.

Data-parallel over batch: each core computes one batch element end-to-end.
No collectives.

Per-core kernel design (cost-model driven):
  - QKV and output projections run as 3-stream hi/lo fp8-e4m3 DoubleRow
    matmuls (xh@Wh + xh@Wl + xl@Wh): K=256 per instruction at 0.5
    cycles/row -> ~4x the fp32r/bf16 GEMM rate. Weights are scaled by 32
    so lo-plane residuals clear the e4m3 subnormal floor; the 1/1024
    compensation folds into the exp scale and the final output copy.
  - S = K^T Q stays bf16 [keys, queries] with exact causal column counts.
  - PV is flipped to [queries, dk] (M fully packed) in bf16 with a ones
    column appended to V for the softmax denominators; normalization is a
    per-partition divide, then a PE transpose back to feature-major for
    the output projection.
  - Weights are pre-split/laid out on the host; engine work is balanced
    across Act (exp), DVE (copies/masks) and Pool (converts/normalize).

Self-contained: builds and compiles the Bass program on first call and
caches it for subsequent calls.
"""

import numpy as np
import ml_dtypes

import concourse.bass as bass
import concourse.mybir as mybir
from concourse import bacc
from concourse.tile import TileContext
from concourse.bass_utils import run_bass_kernel_spmd
from concourse.masks import make_identity, make_upper_triangular

f32 = mybir.dt.float32
f32r = mybir.dt.float32r
bf16 = mybir.dt.bfloat16
e4 = mybir.dt.float8e4
u8 = mybir.dt.uint8
EXP = mybir.ActivationFunctionType.Exp
COPY = mybir.ActivationFunctionType.Copy
DIV = mybir.AluOpType.divide
DR = mybir.MatmulPerfMode.DoubleRow
E4NP = ml_dtypes.float8_e4m3

N_CORES = 8
T = 1024
C = 768
H = 12
DK = 64
NTT = 8            # token tiles of 128
NC2 = 3            # K-chunk pairs of 256
SW = 32.0          # weight scale (power of 2)
SCALE_EXP = (DK ** -0.5) / (SW * SW)
SCALE_OUT = 1.0 / (SW * SW)
STREAMS = ((0, 0), (0, 1), (1, 0))   # (x-plane, w-plane) hi/lo cross terms

# per-head pb column offsets: block ki covers queries [128*ki, 1024)
PB_OFF = [0]
for _ki in range(NTT):
    PB_OFF.append(PB_OFF[-1] + (T - 128 * _ki))
PB_W = PB_OFF[-1]  # 4608


def build_program(qkv_bias: bool, out_bias: bool):
    nc = bacc.Bacc("TRN2", num_devices=N_CORES, debug=False)

    x = nc.dram_tensor("x", [T, C], f32, kind="ExternalInput").ap()
    wq_d = [[nc.dram_tensor(f"wq_{s}{c2}", [128, 2 * 3 * C], u8,
                            kind="ExternalInput").ap()
             for c2 in range(NC2)] for s in range(2)]
    wo_d = [[nc.dram_tensor(f"wo_{s}{c2}", [128, 2 * C], u8,
                            kind="ExternalInput").ap()
             for c2 in range(NC2)] for s in range(2)]
    on_d = nc.dram_tensor("ones12", [128, H], u8 if False else bf16,
                          kind="ExternalInput").ap()
    bq_d = nc.dram_tensor("b_qkv", [3 * C], f32, kind="ExternalInput").ap()
    bo_d = nc.dram_tensor("b_out", [C], f32, kind="ExternalInput").ap()
    y = nc.dram_tensor("y", [T, C], f32, kind="ExternalOutput").ap()

    with TileContext(nc) as tc:
        with tc.tile_pool(name="const", bufs=1) as cpool, \
             tc.tile_pool(name="psn", bufs=1, space="PSUM") as psn:

            ident = cpool.tile([128, 128], bf16, tag="ident")
            nc.gpsimd.memset(ident.bitcast(f32), 0.0)
            make_identity(nc, ident, nomemset=True)
            identf = cpool.tile([128, 128], f32r, tag="identf")
            nc.gpsimd.memset(identf.bitcast(f32), 0.0)
            make_identity(nc, identf, nomemset=True)
            tri = cpool.tile([128, 128], bf16, tag="tri")
            nc.gpsimd.memset(tri.bitcast(f32), 0.0)
            nc.gpsimd.affine_select(
                out=tri, in_=tri, compare_op=mybir.AluOpType.is_gt,
                fill=1.0, base=0, pattern=[[-1, 128]], channel_multiplier=1)

            # ---- persistent SBUF tiles
            wq8 = [[cpool.tile([128, 2 * 3 * C], u8, tag=f"wq{s}{c2}",
                               name=f"wq{s}{c2}")
                    for c2 in range(NC2)] for s in range(2)]
            wo8 = [[cpool.tile([128, 2 * C], u8, tag=f"wo{s}{c2}",
                               name=f"wo{s}{c2}")
                    for c2 in range(NC2)] for s in range(2)]
            xT8 = [cpool.tile([128, 6, T], e4, tag=f"xT8{s}", name=f"xT8{s}")
                   for s in range(2)]
            qkT = [cpool.tile([128, T], bf16, tag=f"qkT{m}", name=f"qkT{m}")
                   for m in range(12)]
            vp = [cpool.tile([128, H, 65], bf16, tag=f"vp{t}", name=f"vp{t}")
                  for t in range(NTT)]
            at8 = [[cpool.tile([128, 2 * T], e4, tag=f"at8{s}{c2}",
                               name=f"at8{s}{c2}")
                    for c2 in range(NC2)] for s in range(2)]

            # named PSUM tiles (manually rotated sub-regions)
            pspv = psn.tile([128, 260], f32, tag="pspv")   # 2 x [*,130] qi slots
            pst = psn.tile([128, 1024], bf16, tag="pst")   # 2 x [*,512] qj slots

            def wqv(s, c2):
                return wq8[s][c2].bitcast(e4).rearrange("p (i m) -> p i m", i=2)

            def wov(s, c2):
                return wo8[s][c2].bitcast(e4).rearrange("p (i m) -> p i m", i=2)

            def atv(s, c2):
                return at8[s][c2].rearrange("p (i m) -> p i m", i=2)

            def x8(s, c2, a, b):
                return xT8[s][:, 2 * c2:2 * c2 + 2, a:b]

            if qkv_bias or out_bias:
                ones_bf = cpool.tile([1, 512], bf16, tag="ones_bf")
                nc.gpsimd.memset(ones_bf, 1.0)
            if qkv_bias:
                bqf = cpool.tile([1, 3 * C], f32, tag="bqf")
                nc.sync.dma_start(out=bqf, in_=bq_d[None, :])
                bqb = cpool.tile([1, 3 * C], bf16, tag="bqb")
                nc.vector.tensor_scalar_mul(bqb, bqf, SW)
            if out_bias:
                bof = cpool.tile([1, C], f32, tag="bof")
                nc.sync.dma_start(out=bof, in_=bo_d[None, :])
                bob = cpool.tile([1, C], bf16, tag="bob")
                nc.vector.tensor_scalar_mul(bob, bof, SW * SW)

            # ---- x load -> PE transpose (f32r) -> fp8 hi/lo split
            # Wv slices ([2C:3C] of each plane) DMA'd on the Act queue,
            # interleaved with x-odd loads; Wqk+Wo on SP after x-evens.
            with tc.tile_pool(name="xst", bufs=3) as xst, \
                 tc.tile_pool(name="xtp", bufs=2, space="PSUM") as xtpp, \
                 tc.tile_pool(name="psv", bufs=2, space="PSUM") as psv:
                for t in range(NTT):
                    ts0, ts1 = t * 128, (t + 1) * 128
                    xs = xst.tile([128, C], f32r, tag="xs")
                    # half-column loads across both DMA queues; Wv plane
                    # DMAs right after the first tile on the Act queue
                    if t == 0:
                        nc.sync.dma_start(out=xs[:, 0:128],
                                          in_=x[ts0:ts1, 0:128].bitcast(f32r))
                        nc.sync.dma_start(out=xs[:, 128:384],
                                          in_=x[ts0:ts1, 128:384].bitcast(f32r))
                    else:
                        nc.sync.dma_start(out=xs[:, 0:384],
                                          in_=x[ts0:ts1, 0:384].bitcast(f32r))
                    xbq = nc.gpsimd if t == 0 else nc.scalar
                    xbq.dma_start(out=xs[:, 384:768],
                                  in_=x[ts0:ts1, 384:768].bitcast(f32r))
                    if t == 0:
                        engs = [nc.sync, nc.scalar, nc.gpsimd]
                        for c2 in range(NC2):
                            for s in range(2):
                                wv3 = wq_d[s][c2].rearrange(
                                    "p (i m) -> p i m", i=2)
                                dst = wq8[s][c2].rearrange(
                                    "p (i m) -> p i m", i=2)[:, :, 2 * C:3 * C]
                                engs[(2 * c2 + s) % 3].dma_start(
                                    out=dst, in_=wv3[:, :, 2 * C:3 * C])
                    for hf in range(2):
                        xtp = xtpp.tile([128, 384], f32r, tag="xtp")
                        for j in range(3):
                            c = 3 * hf + j
                            nc.tensor.transpose(
                                xtp[:, j * 128:(j + 1) * 128],
                                xs[:, c * 128:(c + 1) * 128], identf)
                        src = xtp.bitcast(f32).rearrange(
                            "p (c w) -> p c w", w=128)
                        hi = xT8[0][:, 3 * hf:3 * hf + 3, ts0:ts1]
                        nc.scalar.activation(hi, src, COPY)
                        nc.vector.tensor_sub(
                            xT8[1][:, 3 * hf:3 * hf + 3, ts0:ts1], src, hi)

                    # V'(t) = x(t) @ Wv (3-stream fp8 DR), token-major
                    pv = psv.tile([128, C], f32, tag="pv")
                    for n0, nw in ((0, 512), (512, 256)):
                        first = True
                        if qkv_bias:
                            nc.tensor.matmul(
                                pv[:, n0:n0 + nw], ones_bf[0:1, 0:128],
                                bqb[0:1, 2 * C + n0:2 * C + n0 + nw],
                                start=True, stop=False)
                            first = False
                        for c2 in range(NC2):
                            for sa, sb in STREAMS:
                                last = c2 == NC2 - 1 and (sa, sb) == (1, 0)
                                nc.tensor.matmul(
                                    pv[:, n0:n0 + nw],
                                    x8(sa, c2, ts0, ts1),
                                    wqv(sb, c2)[:, :, 2 * C + n0:
                                                2 * C + n0 + nw],
                                    start=first, stop=last, perf_mode=DR)
                                first = False
                    pvv = pv.rearrange("p (h e) -> p h e", e=64)
                    nc.vector.tensor_copy(vp[t][:, 0:6, 0:64], pvv[:, 0:6, :])
                    nc.scalar.activation(vp[t][:, 6:12, 0:64], pvv[:, 6:12, :],
                                         COPY)

                # Wqk on the Pool (swdge) queue to keep SP/Act free for x
                for c2 in range(NC2):
                    for s in range(2):
                        wv3 = wq_d[s][c2].rearrange("p (i m) -> p i m", i=2)
                        dst = wq8[s][c2].rearrange(
                            "p (i m) -> p i m", i=2)[:, :, 0:2 * C]
                        nc.gpsimd.dma_start(out=dst, in_=wv3[:, :, 0:2 * C])
                # ones columns of V' (softmax denominators); DMA'd because
                # hardware memset cannot write 2-byte value types
                for t in range(NTT):
                    nc.gpsimd.dma_start(out=vp[t][:, :, 64:65],
                                        in_=on_d.rearrange("p (h o) -> p h o",
                                                           o=1))

            # ---- Q,K + attention, interleaved per head pair
            with tc.tile_pool(name="psqk", bufs=2, space="PSUM") as psqkp, \
                 tc.tile_pool(name="pss", bufs=2, space="PSUM") as pss, \
                 tc.tile_pool(name="pb", bufs=2) as pbp, \
                 tc.tile_pool(name="pvs", bufs=6) as pvsp, \
                 tc.tile_pool(name="rc", bufs=6) as rcp, \
                 tc.tile_pool(name="at", bufs=6) as atp:
                carried = []
                for m in range(6):
                    # q features (m), k features (6+m)
                    def emit_qk(mm, nj):
                        pq = psqkp.tile([128, 512], f32, tag="pq", name="pq")
                        first = True
                        if qkv_bias:
                            nc.tensor.matmul(
                                pq, bqb[0:1, mm * 128:(mm + 1) * 128],
                                ones_bf, start=True, stop=False)
                            first = False
                        for c2 in range(NC2):
                            for sa, sb in STREAMS:
                                last = (c2 == NC2 - 1
                                        and (sa, sb) == (1, 0))
                                nc.tensor.matmul(
                                    pq,
                                    wqv(sb, c2)[:, :, mm * 128:(mm + 1) * 128],
                                    x8(sa, c2, nj * 512, (nj + 1) * 512),
                                    start=first, stop=last, perf_mode=DR)
                                first = False
                        nc.vector.tensor_copy(
                            qkT[mm][:, nj * 512:(nj + 1) * 512], pq)

                    if m == 0:      # pair 0: nj=0 q/k before attention
                        emit_qk(0, 0)
                        emit_qk(6, 0)
                    if m == 1:      # Wout loads queue behind Wqk on Pool
                        for c2o in range(NC2):
                            for so in range(2):
                                nc.gpsimd.dma_start(out=wo8[so][c2o],
                                                    in_=wo_d[so][c2o])
                    # qj=0 carries this pair's nj=1 q/k groups (needed only
                    # by qj=1); qj=1 carries the next pair's nj=0 groups.
                    # Both act as PE filler while Act runs exp.
                    nxt0 = [(m, 1), (6 + m, 1)]
                    nxt1 = [(m + 1, 0), (7 + m, 0)] if m < 5 else []

                    # attention for head pair (2m, 2m+1)
                    pbt = pbp.tile([128, 2 * PB_W], bf16, tag="pbt",
                                   name=f"pb{m}")
                    qT, kT = qkT[m], qkT[6 + m]
                    for qj in range(2):
                        nxt = nxt0 if qj == 0 else nxt1
                        for ki in range(4 * qj + 4):
                            if ki % 2 == 1 and nxt:
                                emit_qk(*nxt.pop(0))
                            if m == 5 and qj == 1 and ki in (1, 3):
                                # last pair has no next-pair q/k filler:
                                # start out-proj t=0 groups over c2 0..1
                                n0o = 0 if ki == 1 else 512
                                og = psqkp.tile([128, 512], f32, tag="pq",
                                                name=f"og{ki}")
                                ofirst = True
                                if out_bias:
                                    nc.tensor.matmul(
                                        og[:, 0:512 if n0o == 0 else 256],
                                        ones_bf[0:1, 0:128],
                                        bob[0:1, n0o:n0o + (512 if n0o == 0
                                                            else 256)],
                                        start=True, stop=False)
                                    ofirst = False
                                nwo = 512 if n0o == 0 else 256
                                for c2o in range(2):
                                    for sa, sb in STREAMS:
                                        nc.tensor.matmul(
                                            og[:, 0:nwo],
                                            atv(sa, c2o)[:, :, 0:128],
                                            wov(sb, c2o)[:, :, n0o:n0o + nwo],
                                            start=ofirst, stop=False,
                                            perf_mode=DR)
                                        ofirst = False
                                carried.append((og, n0o, nwo))
                            o = max(0, 128 * ki - 512 * qj)
                            ps = pss.tile([128, 1024], f32, tag="s",
                                          name=f"s{m}_{qj}_{ki}")
                            for e in range(2):
                                nc.tensor.matmul(
                                    ps[:, e * 512 + o:(e + 1) * 512],
                                    kT[64 * e:64 * e + 64,
                                       ki * 128:(ki + 1) * 128],
                                    qT[64 * e:64 * e + 64,
                                       qj * 512 + o:(qj + 1) * 512],
                                    start=True, stop=True)
                            # exp both heads in one strided op
                            wv = 512 - o
                            rel = PB_OFF[ki] + 512 * qj + o - 128 * ki
                            src = ps.rearrange(
                                "p (e w) -> p e w", w=512)[:, :, o:512]
                            dst = pbt.rearrange(
                                "p (e w) -> p e w", w=PB_W)[:, :, rel:rel + wv]
                            nc.scalar.activation(dst, src, EXP,
                                                 scale=float(SCALE_EXP))
                            if ki >= 4 * qj:
                                # diagonal block: causal mask
                                for e in range(2):
                                    dg = pbt[:, e * PB_W + PB_OFF[ki]:
                                             e * PB_W + PB_OFF[ki] + 128]
                                    eng = nc.gpsimd if e == 0 else nc.vector
                                    eng.tensor_mul(dg, dg, tri)
                                # PV(qi=ki) is unblocked as soon as this
                                # block's exp+mask lands: emit it here so
                                # the PE interleaves PV with the next S
                                qi = ki
                                base = (qi % 2) * 130
                                half = ((2 * m + qj) % 2) * 512
                                for e in range(2):
                                    for kj in range(qi + 1):
                                        lo = PB_OFF[kj] + 128 * (qi - kj)
                                        nc.tensor.matmul(
                                            pspv[:, base + e * 65:
                                                 base + (e + 1) * 65],
                                            pbt[:, e * PB_W + lo:
                                                e * PB_W + lo + 128],
                                            vp[kj][:, 2 * m + e:2 * m + e + 1, :],
                                            start=(kj == 0), stop=(kj == qi))
                                if qi % 2 == 1:
                                    # both slots full: one copy + recip for
                                    # the qi pair, then normalize+transpose
                                    pvc = pvsp.tile([128, 260], f32,
                                                    tag="pvc")
                                    nc.vector.tensor_copy(pvc, pspv)
                                    rc = rcp.tile([128, 4], f32, tag="rc")
                                    nc.vector.reciprocal(
                                        rc.rearrange("p (e w) -> p e w", w=1),
                                        pvc.rearrange("p (e w) -> p e w",
                                                      w=65)[:, :, 64:65])
                                    for qq in (qi - 1, qi):
                                        b2 = (qq % 2) * 130
                                        at = atp.tile([128, 128], bf16,
                                                      tag="at")
                                        for e in range(2):
                                            nc.gpsimd.tensor_scalar_mul(
                                                at[:, e * 64:(e + 1) * 64],
                                                pvc[:, b2 + e * 65:
                                                    b2 + e * 65 + 64],
                                                rc[:, (qq % 2) * 2 + e:
                                                   (qq % 2) * 2 + e + 1])
                                        col = half + (qq - 4 * qj) * 128
                                        nc.tensor.transpose(
                                            pst[:, col:col + 128], at, ident)
                        half = ((2 * m + qj) % 2) * 512
                        while nxt:          # flush remaining q/k groups
                            emit_qk(*nxt.pop(0))
                        # feature-major fp8 hi/lo of this qj's attn
                        c2, i = m // 2, m % 2
                        hi = at8[0][c2][:, i * T + qj * 512:i * T + (qj + 1) * 512]
                        nc.vector.tensor_copy(hi, pst[:, half:half + 512])
                        nc.vector.tensor_sub(
                            at8[1][c2][:, i * T + qj * 512:i * T + (qj + 1) * 512],
                            pst[:, half:half + 512], hi)

                for og, n0, nw in carried:      # finish pair-5-carried t=0
                    for sa, sb in STREAMS:
                        nc.tensor.matmul(
                            og[:, 0:nw], atv(sa, 2)[:, :, 0:128],
                            wov(sb, 2)[:, :, n0:n0 + nw],
                            start=False, stop=(sa, sb) == (1, 0),
                            perf_mode=DR)
                    ys = atp.tile([128, 512], f32, tag="ysc", name="ysc")
                    nc.vector.tensor_scalar_mul(ys[:, 0:nw], og[:, 0:nw],
                                                float(SCALE_OUT))
                    deng = nc.sync if n0 == 0 else nc.scalar
                    deng.dma_start(out=y[0:128, n0:n0 + nw], in_=ys[:, 0:nw])

            # ---- output projection: y = attn' @ Wout' / 1024
            with tc.tile_pool(name="pso", bufs=3, space="PSUM") as pso, \
                 tc.tile_pool(name="yst", bufs=4) as yst:
                for t in range(NTT):
                    for n0, nw in ((0, 512), (512, 256)):
                        if t == 0 and carried:
                            continue
                        pot = pso.tile([128, 512], f32, tag="po", name="po")
                        po = pot[:, 0:nw]
                        first = True
                        if out_bias:
                            nc.tensor.matmul(
                                po, ones_bf[0:1, 0:128],
                                bob[0:1, n0:n0 + nw], start=True, stop=False)
                            first = False
                        for c2 in range(NC2):
                            for sa, sb in STREAMS:
                                last = c2 == NC2 - 1 and (sa, sb) == (1, 0)
                                nc.tensor.matmul(
                                    po, atv(sa, c2)[:, :, t * 128:(t + 1) * 128],
                                    wov(sb, c2)[:, :, n0:n0 + nw],
                                    start=first, stop=last, perf_mode=DR)
                                first = False
                        ys = yst.tile([128, 512], f32, tag="ys")
                        nc.vector.tensor_scalar_mul(ys[:, 0:nw], po,
                                                    float(SCALE_OUT))
                        deng = nc.sync if (2 * t + n0 // 512) % 2 == 0 \
                            else nc.scalar
                        deng.dma_start(
                            out=y[t * 128:(t + 1) * 128, n0:n0 + nw],
                            in_=ys[:, 0:nw])

    nc.compile()
    return nc


_CACHE = {}


def _get_program(qkv_bias: bool, out_bias: bool):
    key = (qkv_bias, out_bias)
    if key not in _CACHE:
        _CACHE[key] = build_program(qkv_bias, out_bias)
    return _CACHE[key]


def prep_weights(W_qkv, W_out):
    """Host-side: scale by 32, split into e4m3 hi/lo, DoubleRow layout
    [128, 2, M] per 256-wide K chunk, as raw bytes."""
    out = {}
    for name, W, M in (("wq", np.asarray(W_qkv, np.float32), 3 * C),
                       ("wo", np.asarray(W_out, np.float32), C)):
        Ws = W * SW
        hi = Ws.astype(E4NP)
        lo = (Ws - hi.astype(np.float32)).astype(E4NP)
        for s, plane in enumerate((hi, lo)):
            a = plane.reshape(NC2, 2, 128, M)          # [c2, i, p, m]
            for c2 in range(NC2):
                lay = np.ascontiguousarray(
                    a[c2].transpose(1, 0, 2))          # [p, i, m]
                out[f"{name}_{s}{c2}"] = lay.reshape(128, 2 * M).view(np.uint8)
    return out


def _make_in_maps(x, W_qkv, b_qkv, W_out, b_out):
    x = np.ascontiguousarray(np.asarray(x, dtype=np.float32))
    b_qkv = np.ascontiguousarray(np.asarray(b_qkv, dtype=np.float32))
    b_out = np.ascontiguousarray(np.asarray(b_out, dtype=np.float32))
    w = prep_weights(W_qkv, W_out)
    w["ones12"] = np.ones((128, H), dtype=ml_dtypes.bfloat16)
    return [
        {"x": x[i], "b_qkv": b_qkv, "b_out": b_out, **w}
        for i in range(N_CORES)
    ]


def kernel(x, W_qkv, b_qkv, W_out, b_out):
    qkv_bias = bool(np.any(np.asarray(b_qkv)))
    out_bias = bool(np.any(np.asarray(b_out)))
    nc = _get_program(qkv_bias, out_bias)
    in_maps = _make_in_maps(x, W_qkv, b_qkv, W_out, b_out)
    res = run_bass_kernel_spmd(nc, in_maps, core_ids=list(range(N_CORES)))
    return np.stack([res.results[i]["y"] for i in range(N_CORES)], axis=0)


def bench(x, W_qkv, b_qkv, W_out, b_out, trace=True):
    """Run with NTFF tracing; returns (output, BassKernelResults)."""
    qkv_bias = bool(np.any(np.asarray(b_qkv)))
    out_bias = bool(np.any(np.asarray(b_out)))
    nc = _get_program(qkv_bias, out_bias)
    in_maps = _make_in_maps(x, W_qkv, b_qkv, W_out, b_out)
    res = run_bass_kernel_spmd(nc, in_maps, core_ids=list(range(N_CORES)),
                               trace=trace)
    out = np.stack([res.results[i]["y"] for i in range(N_CORES)], axis=0)
    return out, res
